# revision 5
# baseline (speedup 1.0000x reference)
"""Causal self-attention with RoPE on 8 TRN2 NeuronCores.

Sharding: 2 (batch) x 4 (head-group tensor parallel). Core c handles
batch b=c//4 and heads [4g, 4g+4) with g=c%4. Each core computes
q,k,v projections for its heads, RoPE, causal attention, and its
partial of the output projection; the host sums the 4 partials per
batch (the "all-reduce").

Self-contained: hardcodes shapes from the problem spec.
"""
import numpy as np
import ml_dtypes

import concourse.bass as bass
import concourse.mybir as mybir
import concourse.tile as tile
from concourse import bacc
from concourse.bass_utils import run_bass_kernel_spmd

F32 = mybir.dt.float32
F32R = mybir.dt.float32r
BF16 = mybir.dt.bfloat16

B, T, DIM = 2, 2048, 1024
HEADS, HEAD_DIM = 16, 64
INNER = HEADS * HEAD_DIM
ROPE_BASE = 10000.0
N_CORES = 8
TPG = 4                      # tensor-parallel group size (head groups)
HPC = HEADS // TPG           # heads per core = 4
LOC = HPC * HEAD_DIM         # local inner = 256
SCALE = 1.0 / np.sqrt(HEAD_DIM)

TB = 512                     # t block for QKV / q block for attention
NTB = T // TB                # 4
NKT = T // 128               # 16 k tiles
ND = DIM // 128              # 8 contraction chunks


def _host_constants():
    inv_freq = 1.0 / (ROPE_BASE ** (np.arange(0, HEAD_DIM, 2, dtype=np.float32) / HEAD_DIM))
    t = np.arange(T, dtype=np.float32)
    freqs = np.outer(t, inv_freq).astype(np.float32)          # [T, 32]
    cos32 = np.cos(freqs).T.astype(np.float32)                # [32, T]
    sin32 = np.sin(freqs).T.astype(np.float32)
    cosT = np.tile(cos32, (4, 1))                             # [128, T]
    sinT = np.tile(sin32, (4, 1))

    # rot matrix: rot[m] = -x[m+32] (m%64<32), +x[m-32] (m%64>=32); lhsT[k, m]
    prot = np.zeros((128, 128), dtype=np.float32)
    for blk in range(2):
        o = blk * 64
        for m in range(32):
            prot[o + m + 32, o + m] = -1.0
            prot[o + m, o + m + 32] = 1.0

    # post-exp 0/1 causal mask for the diagonal 128-col block: keep j >= p
    j = np.arange(128)[None, :]
    p = np.arange(128)[:, None]
    mask01 = (j >= p).astype(ml_dtypes.bfloat16)              # [128, 128]
    return cosT, sinT, prot, mask01


def build_kernel(tc):
    nc = tc.nc
    xT = nc.dram_tensor("xT", [DIM, T], BF16, kind="ExternalInput").ap()
    w_qk = nc.dram_tensor("w_qk", [DIM, 2 * LOC], BF16, kind="ExternalInput").ap()
    w_v = nc.dram_tensor("w_v", [DIM, LOC], BF16, kind="ExternalInput").ap()
    w_pr = nc.dram_tensor("w_pr", [LOC, DIM], BF16, kind="ExternalInput").ap()
    cosT_d = nc.dram_tensor("cosT", [128, T], F32, kind="ExternalInput").ap()
    sinT_d = nc.dram_tensor("sinT", [128, T], F32, kind="ExternalInput").ap()
    prot_d = nc.dram_tensor("prot", [128, 128], BF16, kind="ExternalInput").ap()
    mask01_d = nc.dram_tensor("mask01", [128, 128], BF16, kind="ExternalInput").ap()
    out_d = nc.dram_tensor("out", [T, DIM], F32, kind="ExternalOutput").ap()

    with (
        tc.tile_pool(name="const", bufs=1) as const,
        tc.tile_pool(name="xt", bufs=2) as xt_pool,
        tc.tile_pool(name="persist", bufs=1) as persist,
        tc.tile_pool(name="work", bufs=3) as work,
        tc.tile_pool(name="expp", bufs=2) as expp,
        tc.tile_pool(name="ps_sc", bufs=1, space="PSUM") as ps_sc,
        tc.tile_pool(name="ps_acc", bufs=1, space="PSUM") as ps_acc,
        tc.tile_pool(name="ps_mm", bufs=1, space="PSUM") as ps_mm,
        tc.tile_pool(name="dram", bufs=2, space="DRAM") as dram,
    ):
        # ---- constants / weights ----
        wqk_sb = []
        wv_sb = []
        for d in range(ND):
            wq_t = const.tile([128, 2 * LOC], BF16, tag=f"wqk{d}", name=f"wqk{d}")
            nc.sync.dma_start(out=wq_t, in_=w_qk[d * 128:(d + 1) * 128, :])
            wqk_sb.append(wq_t)
            wv_t = const.tile([128, LOC], BF16, tag=f"wv{d}", name=f"wv{d}")
            nc.sync.dma_start(out=wv_t, in_=w_v[d * 128:(d + 1) * 128, :])
            wv_sb.append(wv_t)
        wpr_sb = []
        for c in range(2):
            wp_t = const.tile([128, DIM], BF16, tag=f"wpr{c}", name=f"wpr{c}")
            nc.sync.dma_start(out=wp_t, in_=w_pr[c * 128:(c + 1) * 128, :])
            wpr_sb.append(wp_t)
        cos_sb = const.tile([128, T], F32, tag="cos")
        nc.sync.dma_start(out=cos_sb, in_=cosT_d)
        sin_sb = const.tile([128, T], F32, tag="sin")
        nc.sync.dma_start(out=sin_sb, in_=sinT_d)
        prot_sb = const.tile([128, 128], BF16, tag="prot")
        nc.sync.dma_start(out=prot_sb, in_=prot_d)
        mask_sb = const.tile([128, 128], BF16, tag="mask")
        nc.sync.dma_start(out=mask_sb, in_=mask01_d)
        ones_sb = const.tile([128, 1], BF16, tag="ones")
        nc.vector.memset(ones_sb, 1.0)

        # ---- phase 1: QKV + RoPE ----
        # qk_rope[p][tb]: pair p (heads 2p,2p+1 on partitions [0:64],[64:128]);
        # m=0,1 -> q pairs, m=2,3 -> k pairs
        qk_rope = [[persist.tile([128, TB], BF16, tag=f"qkr{m}_{tb}", name=f"qkr{m}_{tb}")
                    for tb in range(NTB)] for m in range(4)]
        v_sb = [persist.tile([128, LOC], BF16, tag=f"v{ts}", name=f"v{ts}") for ts in range(NKT)]

        for tb in range(NTB):
            xt_sb = []
            for d in range(ND):
                x_t = xt_pool.tile([128, TB], BF16, tag=f"xt{d}")
                nc.sync.dma_start(out=x_t, in_=xT[d * 128:(d + 1) * 128, tb * TB:(tb + 1) * TB])
                xt_sb.append(x_t)
            qk4_ps = ps_sc.tile([128, 4, TB], F32, tag="sc", name=f"qk4_{tb}")
            for m in range(4):
                for d in range(ND):
                    nc.tensor.matmul(
                        qk4_ps[:, m, :],
                        lhsT=wqk_sb[d][:, m * 128:(m + 1) * 128],
                        rhs=xt_sb[d],
                        start=(d == 0), stop=(d == ND - 1),
                    )
                raw_sb = work.tile([128, TB], BF16, tag="raw")
                nc.vector.tensor_copy(raw_sb, qk4_ps[:, m, :])
                rot_ps = ps_mm.tile([128, TB], F32, tag="mm", name="rot_ps")
                nc.tensor.matmul(rot_ps, lhsT=prot_sb,
                                 rhs=raw_sb, start=True, stop=True)
                qc_sb = work.tile([128, TB], F32, tag="qc")
                nc.vector.tensor_mul(qc_sb, raw_sb, cos_sb[:, tb * TB:(tb + 1) * TB])
                rs_sb = work.tile([128, TB], F32, tag="rs")
                nc.vector.tensor_mul(rs_sb, rot_ps, sin_sb[:, tb * TB:(tb + 1) * TB])
                nc.vector.tensor_add(qk_rope[m][tb], qc_sb, rs_sb)
            for s in range(4):
                ts = tb * 4 + s
                v_ps = ps_mm.tile([128, LOC], F32, tag="mm", name="v_ps")
                for d in range(ND):
                    nc.tensor.matmul(
                        v_ps,
                        lhsT=xt_sb[d][:, s * 128:(s + 1) * 128],
                        rhs=wv_sb[d],
                        start=(d == 0), stop=(d == ND - 1),
                    )
                nc.vector.tensor_copy(v_sb[ts], v_ps)

        # ---- phase 2: attention (scoresT layout) + phase 3 proj per qb ----
        for qb in range(NTB):
            nkt = 4 * (qb + 1)
            av_ps = [ps_acc.tile([128, TB], F32, tag=f"av{p}", name=f"av{p}") for p in range(2)]
            rsum_ps = ps_acc.tile([128, TB], F32, tag="rsum")
            sc_ps = ps_sc.tile([128, 4, TB], F32, tag="sc", name=f"sc{qb}")
            exp_sb = expp.tile([128, 4, TB], BF16, tag="exp")
            for kt in range(nkt):
                ktl = kt - 4 * qb
                a = 128 * ktl if ktl >= 0 else 0
                w = TB - a
                tbk, ok = kt // 4, (kt % 4) * 128
                for p in range(2):
                    for j in range(2):
                        h = 2 * p + j
                        nc.tensor.matmul(
                            sc_ps[:, h, 0:w],
                            lhsT=qk_rope[2 + p][tbk][64 * j:64 * j + 64, ok:ok + 128],
                            rhs=qk_rope[p][qb][64 * j:64 * j + 64, a:TB],
                            start=True, stop=True, tile_position=(64 * j, 0),
                        )
                nc.scalar.activation(exp_sb[:, :, 0:w], sc_ps[:, :, 0:w],
                                     mybir.ActivationFunctionType.Exp, scale=float(SCALE))
                if ktl >= 0:
                    nc.vector.tensor_mul(
                        exp_sb[:, :, 0:128], exp_sb[:, :, 0:128],
                        mask_sb.rearrange("p (o n) -> p o n", o=1).to_broadcast([128, 4, 128]),
                    )
                for p in range(2):
                    for j in range(2):
                        h = 2 * p + j
                        nc.tensor.matmul(
                            av_ps[p][64 * j:64 * j + 64, a:TB],
                            lhsT=v_sb[kt][:, 64 * h:64 * h + 64],
                            rhs=exp_sb[:, h, 0:w],
                            start=(kt == 0), stop=(kt == nkt - 1),
                            skip_group_check=True,
                            tile_position=(0, 64 * j),
                        )
                for h in range(4):
                    nc.tensor.matmul(
                        rsum_ps[32 * h:32 * h + 1, a:TB],
                        lhsT=ones_sb,
                        rhs=exp_sb[:, h, 0:w],
                        start=(kt == 0), stop=(kt == nkt - 1),
                        skip_group_check=True,
                        tile_position=(0, 32 * h),
                    )
            # reciprocal + DRAM-roundtrip broadcast of 1/rowsum
            recip_sb = work.tile([128, TB], F32, tag="recip")
            for h in range(4):
                nc.vector.reciprocal(recip_sb[32 * h:32 * h + 1, :],
                                     rsum_ps[32 * h:32 * h + 1, :])
            recip_dr = dram.tile([4, TB], F32, tag="recip_dr")
            nc.sync.dma_start(
                out=recip_dr,
                in_=recip_sb.rearrange("(a b) n -> a b n", b=32)[:, 0:1, :],
            )
            outT_sb = [persist.tile([128, TB], BF16, tag=f"outT{qb}_{p}", name=f"outT{qb}_{p}") for p in range(2)]
            for p in range(2):
                for j in range(2):
                    h = 2 * p + j
                    rb = work.tile([64, TB], F32, tag="recip_bc")
                    nc.gpsimd.dma_start(
                        out=rb,
                        in_=bass.AP(tensor=recip_dr.tensor,
                                    offset=recip_dr.offset + h * TB,
                                    ap=[[0, 64], [1, TB]]),
                    )
                    nc.vector.tensor_mul(outT_sb[p][64 * j:64 * j + 64, :],
                                         av_ps[p][64 * j:64 * j + 64, :], rb)
            # proj for this qb's 4 t-subtiles
            for s in range(4):
                for n in range(2):
                    pr_ps = ps_mm.tile([128, TB], F32, tag="mm", name="pr_ps")
                    for p in range(2):
                        nc.tensor.matmul(
                            pr_ps,
                            lhsT=outT_sb[p][:, s * 128:(s + 1) * 128],
                            rhs=wpr_sb[p][:, n * TB:(n + 1) * TB],
                            start=(p == 0), stop=(p == 1),
                        )
                    pr_sb = work.tile([128, TB], F32, tag="pr_sb")
                    nc.vector.tensor_copy(pr_sb, pr_ps)
                    nc.sync.dma_start(
                        out=out_d[(qb * 4 + s) * 128:(qb * 4 + s + 1) * 128,
                                  n * TB:(n + 1) * TB],
                        in_=pr_sb,
                    )


def shard_inputs(x, w_qkv, w_proj):
    """Full inputs -> list of 8 per-core input maps."""
    cosT, sinT, prot, mask01 = _host_constants()
    x = np.ascontiguousarray(np.asarray(x, dtype=np.float32))
    w_qkv = np.asarray(w_qkv, dtype=np.float32)
    w_proj = np.asarray(w_proj, dtype=np.float32)
    in_maps = []
    for c in range(N_CORES):
        b, g = c // TPG, c % TPG
        xT = np.ascontiguousarray(x[b].T)                     # [DIM, T]
        wq = w_qkv[:, g * LOC:(g + 1) * LOC]
        wk = w_qkv[:, INNER + g * LOC:INNER + (g + 1) * LOC]
        wv = w_qkv[:, 2 * INNER + g * LOC:2 * INNER + (g + 1) * LOC]
        w_qk = np.ascontiguousarray(np.concatenate([wq, wk], axis=1))  # [DIM, 512]
        w_pr = np.ascontiguousarray(w_proj[g * LOC:(g + 1) * LOC, :])  # [256, DIM]
        in_maps.append({
            "xT": xT.astype(ml_dtypes.bfloat16),
            "w_qk": w_qk.astype(ml_dtypes.bfloat16),
            "w_v": np.ascontiguousarray(wv).astype(ml_dtypes.bfloat16),
            "w_pr": w_pr.astype(ml_dtypes.bfloat16),
            "cosT": cosT,
            "sinT": sinT,
            "prot": prot.astype(ml_dtypes.bfloat16),
            "mask01": mask01,
        })
    return in_maps


_CACHE = {}


def _get_compiled():
    if "nc" not in _CACHE:
        nc = bacc.Bacc("TRN2", target_bir_lowering=False, debug=False,
                       enable_asserts=True, num_devices=N_CORES)
        with tile.TileContext(nc) as tc:
            build_kernel(tc)
        nc.compile()
        _CACHE["nc"] = nc
    return _CACHE["nc"]


def kernel(x, w_qkv, w_proj):
    nc = _get_compiled()
    in_maps = shard_inputs(x, w_qkv, w_proj)
    res = run_bass_kernel_spmd(nc, in_maps, core_ids=list(range(N_CORES)))
    outs = [res.results[c]["out"] for c in range(N_CORES)]
    full = np.stack([
        np.sum([outs[b * TPG + g] for g in range(TPG)], axis=0, dtype=np.float32)
        for b in range(B)
    ])
    return full.astype(np.float32)


# revision 11
# speedup vs baseline: 1.5470x; 1.5470x over previous
"""Causal self-attention with RoPE on 8 TRN2 NeuronCores.

Sharding: 2 (batch) x 4 (head-group tensor parallel). Core c handles
batch b=c//4 and heads [4g, 4g+4) with g=c%4. Each core computes its
q,k,v projections, RoPE, causal attention (transposed-scores flash
layout), and its partial of the output projection; the host sums the
4 partials per batch (the "all-reduce").

v2: fused per-t-block loop (QKV(tb) -> attention(qb=tb) -> proj(qb))
so PE stays dense and ACT/DVE overlap; pair-split double-buffered
score chunks; batched approximate reciprocal for softmax norms.

Self-contained: hardcodes shapes from the problem spec.
"""
import numpy as np
import ml_dtypes

import concourse.bass as bass
import concourse.mybir as mybir
import concourse.tile as tile
from concourse import bacc
from concourse.bass_utils import run_bass_kernel_spmd

F32 = mybir.dt.float32
BF16 = mybir.dt.bfloat16

B, T, DIM = 2, 2048, 1024
HEADS, HEAD_DIM = 16, 64
INNER = HEADS * HEAD_DIM
ROPE_BASE = 10000.0
N_CORES = 8
TPG = 4                      # tensor-parallel group size (head groups)
HPC = HEADS // TPG           # heads per core = 4
LOC = HPC * HEAD_DIM         # local inner = 256
SCALE = 1.0 / np.sqrt(HEAD_DIM)

TB = 512                     # t block for QKV / q block for attention
NTB = T // TB                # 4
NKT = T // 128               # 16 k tiles
ND = DIM // 128              # 8 contraction chunks


def _host_constants():
    inv_freq = 1.0 / (ROPE_BASE ** (np.arange(0, HEAD_DIM, 2, dtype=np.float32) / HEAD_DIM))
    t = np.arange(T, dtype=np.float32)
    freqs = np.outer(t, inv_freq).astype(np.float32)          # [T, 32]
    cos32 = np.cos(freqs).T.astype(np.float32)                # [32, T]
    sin32 = np.sin(freqs).T.astype(np.float32)
    cosT = np.tile(cos32, (4, 1))                             # [128, T]
    sinT = np.tile(sin32, (4, 1))

    # rot matrix: rot[m] = -x[m+32] (m%64<32), +x[m-32] (m%64>=32); lhsT[k, m]
    prot = np.zeros((128, 128), dtype=np.float32)
    for blk in range(2):
        o = blk * 64
        for m in range(32):
            prot[o + m + 32, o + m] = -1.0
            prot[o + m, o + m + 32] = 1.0

    # post-exp 0/1 causal mask for the diagonal 128-col block: keep j >= p
    j = np.arange(128)[None, :]
    p = np.arange(128)[:, None]
    mask01 = (j >= p).astype(ml_dtypes.bfloat16)              # [128, 128]
    return cosT, sinT, prot, mask01


def build_kernel(tc):
    nc = tc.nc
    xT = nc.dram_tensor("xT", [DIM, T], BF16, kind="ExternalInput").ap()
    w_qk = nc.dram_tensor("w_qk", [DIM, 2 * LOC], BF16, kind="ExternalInput").ap()
    w_v = nc.dram_tensor("w_v", [DIM, LOC], BF16, kind="ExternalInput").ap()
    w_pr = nc.dram_tensor("w_pr", [LOC, DIM], BF16, kind="ExternalInput").ap()
    cosT_d = nc.dram_tensor("cosT", [128, T], F32, kind="ExternalInput").ap()
    sinT_d = nc.dram_tensor("sinT", [128, T], F32, kind="ExternalInput").ap()
    prot_d = nc.dram_tensor("prot", [128, 128], BF16, kind="ExternalInput").ap()
    mask01_d = nc.dram_tensor("mask01", [128, 128], BF16, kind="ExternalInput").ap()
    out_d = nc.dram_tensor("out", [T, DIM], F32, kind="ExternalOutput").ap()

    with (
        tc.tile_pool(name="const", bufs=1) as const,
        tc.tile_pool(name="xt", bufs=2) as xt_pool,
        tc.tile_pool(name="persist", bufs=1) as persist,
        tc.tile_pool(name="work", bufs=3) as work,
        tc.tile_pool(name="expp", bufs=4) as expp,
        tc.tile_pool(name="ps_sc", bufs=2, space="PSUM") as ps_sc,
        tc.tile_pool(name="ps_acc", bufs=1, space="PSUM") as ps_acc,
        tc.tile_pool(name="ps_mm", bufs=1, space="PSUM") as ps_mm,
        tc.tile_pool(name="dram", bufs=2, space="DRAM") as dram,
    ):
        # ---- tb=0 activations + qk weights first: PE starts ASAP ----
        xt_sb = {}
        wqk_sb = []
        for d in range(ND):
            x_t = xt_pool.tile([128, TB], BF16, tag=f"xt{d}", name=f"xt{d}_0")
            nc.sync.dma_start(out=x_t, in_=xT[d * 128:(d + 1) * 128, 0:TB])
            xt_sb[(0, d)] = x_t
            wq_t = const.tile([128, 2 * LOC], BF16, tag=f"wqk{d}", name=f"wqk{d}")
            nc.sync.dma_start(out=wq_t, in_=w_qk[d * 128:(d + 1) * 128, :])
            wqk_sb.append(wq_t)
        prot_sb = const.tile([128, 128], BF16, tag="prot")
        nc.sync.dma_start(out=prot_sb, in_=prot_d)
        cos_sb = const.tile([128, T], F32, tag="cos")
        nc.sync.dma_start(out=cos_sb, in_=cosT_d)
        sin_sb = const.tile([128, T], F32, tag="sin")
        nc.sync.dma_start(out=sin_sb, in_=sinT_d)
        wv_sb = []
        for d in range(ND):
            wv_t = const.tile([128, LOC], BF16, tag=f"wv{d}", name=f"wv{d}")
            nc.sync.dma_start(out=wv_t, in_=w_v[d * 128:(d + 1) * 128, :])
            wv_sb.append(wv_t)
        mask_sb = const.tile([128, 128], BF16, tag="mask")
        nc.sync.dma_start(out=mask_sb, in_=mask01_d)
        ones_sb = const.tile([128, 1], BF16, tag="ones")
        nc.vector.memset(ones_sb, 1.0)
        ones2_sb = const.tile([128, 64], F32, tag="ones2")
        nc.vector.memset(ones2_sb, 1.0)
        wpr_sb = []
        for c in range(2):
            wp_t = const.tile([128, DIM], BF16, tag=f"wpr{c}", name=f"wpr{c}")
            nc.sync.dma_start(out=wp_t, in_=w_pr[c * 128:(c + 1) * 128, :])
            wpr_sb.append(wp_t)

        # persistent per-phase outputs
        qk_rope = [[persist.tile([128, TB], BF16, tag=f"qkr{m}_{tb}", name=f"qkr{m}_{tb}")
                    for tb in range(NTB)] for m in range(4)]
        v_sb = [persist.tile([128, LOC], BF16, tag=f"v{ts}", name=f"v{ts}")
                for ts in range(NKT)]

        mask_bc = mask_sb.rearrange("p (o n) -> p o n", o=1).to_broadcast([128, 2, 128])

        for tb in range(NTB):
            # prefetch next tb's activations (bufs=2 -> depth-1 prefetch)
            if tb > 0:
                for d in range(ND):
                    x_t = xt_pool.tile([128, TB], BF16, tag=f"xt{d}", name=f"xt{d}_{tb}")
                    nc.sync.dma_start(
                        out=x_t, in_=xT[d * 128:(d + 1) * 128, tb * TB:(tb + 1) * TB])
                    xt_sb[(tb, d)] = x_t

            # ---- QKV + RoPE for this t block ----
            for m in range(4):  # m 0,1 -> q pairs; 2,3 -> k pairs
                qk1 = (ps_mm.tile([128, TB], F32, tag="mm", name=f"qk1_{m}_{tb}")
                       if m % 2 == 0 else
                       ps_acc.tile([128, TB], F32, tag="av1", name=f"qk1_{m}_{tb}"))
                for d in range(ND):
                    nc.tensor.matmul(
                        qk1,
                        lhsT=wqk_sb[d][:, m * 128:(m + 1) * 128],
                        rhs=xt_sb[(tb, d)],
                        start=(d == 0), stop=(d == ND - 1),
                    )
                raw_sb = work.tile([128, TB], BF16, tag="raw")
                nc.vector.tensor_copy(raw_sb, qk1)
                rot_ps = ps_acc.tile([128, TB], F32, tag="av0", name=f"rot_{m}_{tb}")
                nc.tensor.matmul(rot_ps, lhsT=prot_sb, rhs=raw_sb,
                                 start=True, stop=True)
                qc_sb = work.tile([128, TB], F32, tag="qc")
                nc.vector.tensor_mul(qc_sb, raw_sb, cos_sb[:, tb * TB:(tb + 1) * TB])
                rs_sb = work.tile([128, TB], F32, tag="rs")
                nc.vector.tensor_mul(rs_sb, rot_ps, sin_sb[:, tb * TB:(tb + 1) * TB])
                nc.vector.tensor_add(qk_rope[m][tb], qc_sb, rs_sb)
            for s in range(4):
                ts = tb * 4 + s
                v_ps = ps_mm.tile([128, LOC], F32, tag="mm", name=f"v_ps{ts}")
                for d in range(ND):
                    nc.tensor.matmul(
                        v_ps,
                        lhsT=xt_sb[(tb, d)][:, s * 128:(s + 1) * 128],
                        rhs=wv_sb[d],
                        start=(d == 0), stop=(d == ND - 1),
                    )
                nc.vector.tensor_copy(v_sb[ts], v_ps)

            # ---- attention for qb=tb (scoresT layout, flash over kt) ----
            qb = tb
            nkt = 4 * (qb + 1)
            av_ps = [ps_acc.tile([128, TB], F32, tag=f"av{p}", name=f"av{p}_{qb}")
                     for p in range(2)]
            rsum_ps = ps_acc.tile([128, TB], F32, tag="rsum", name=f"rsum_{qb}")
            for kt in range(nkt):
                ktl = kt - 4 * qb
                a = 128 * ktl if ktl >= 0 else 0
                w = TB - a
                tbk, ok = kt // 4, (kt % 4) * 128
                for p in range(2):
                    sc2 = ps_sc.tile([128, 2, TB], F32, tag="sc", name=f"sc{qb}_{kt}_{p}")
                    for j in range(2):
                        nc.tensor.matmul(
                            sc2[:, j, 0:w],
                            lhsT=qk_rope[2 + p][tbk][64 * j:64 * j + 64, ok:ok + 128],
                            rhs=qk_rope[p][qb][64 * j:64 * j + 64, a:TB],
                            start=True, stop=True, tile_position=(64 * j, 0),
                        )
                    exp2 = expp.tile([128, 2, TB], BF16, tag="exp", name=f"exp{qb}_{kt}_{p}")
                    nc.scalar.activation(exp2[:, :, 0:w], sc2[:, :, 0:w],
                                         mybir.ActivationFunctionType.Exp,
                                         scale=float(SCALE))
                    if ktl >= 0:
                        nc.vector.tensor_mul(exp2[:, :, 0:128], exp2[:, :, 0:128],
                                             mask_bc)
                    for j in range(2):
                        h = 2 * p + j
                        nc.tensor.matmul(
                            av_ps[p][64 * j:64 * j + 64, a:TB],
                            lhsT=v_sb[kt][:, 64 * h:64 * h + 64],
                            rhs=exp2[:, j, 0:w],
                            start=(kt == 0), stop=(kt == nkt - 1),
                            skip_group_check=True,
                            tile_position=(0, 64 * j),
                        )
                    for j in range(2):
                        h = 2 * p + j
                        nc.tensor.matmul(
                            rsum_ps[32 * h:32 * h + 1, a:TB],
                            lhsT=ones_sb,
                            rhs=exp2[:, j, 0:w],
                            start=(kt == 0), stop=(kt == nkt - 1),
                            skip_group_check=True,
                            tile_position=(0, 32 * h),
                        )

            # softmax normalizer: copy rowsums to SBUF, replicate via K=1
            # outer-product matmuls, then approx reciprocal on the dense tile
            rsum_sb = work.tile([128, TB], F32, tag="recip")
            nc.vector.tensor_copy(rsum_sb, rsum_ps)
            bc2 = ps_sc.tile([128, 2, TB], F32, tag="sc", name=f"bc{qb}")
            for p in range(2):
                for j in range(2):
                    h = 2 * p + j
                    nc.tensor.matmul(
                        bc2[64 * j:64 * j + 64, p, :],
                        lhsT=ones2_sb[32 * h:32 * h + 1, :],
                        rhs=rsum_sb[32 * h:32 * h + 1, :],
                        start=True, stop=True, skip_group_check=True,
                        tile_position=(32 * h, 64 * j),
                    )
            recip2_sb = work.tile([128, 2, TB], F32, tag="recipb")
            scr_sb = work.tile([128, 2, TB], F32, tag="rscr")
            nc.vector.reciprocal_approx_accurate(
                out=recip2_sb, in_=bc2, scratch=scr_sb)
            outT_sb = []
            for p in range(2):
                o_t = persist.tile([128, TB], BF16, tag=f"outT{qb}_{p}", name=f"outT{qb}_{p}")
                nc.vector.tensor_mul(o_t, av_ps[p], recip2_sb[:, p, :])
                outT_sb.append(o_t)

            # ---- output projection for this qb's 4 t-subtiles ----
            for s in range(4):
                for n in range(2):
                    pr2 = ps_sc.tile([128, 2, TB], F32, tag="sc", name=f"pr{qb}_{s}_{n}")
                    pr_ps = pr2[:, 0, :]
                    for p in range(2):
                        nc.tensor.matmul(
                            pr_ps,
                            lhsT=outT_sb[p][:, s * 128:(s + 1) * 128],
                            rhs=wpr_sb[p][:, n * TB:(n + 1) * TB],
                            start=(p == 0), stop=(p == 1),
                        )
                    pr_sb = work.tile([128, TB], F32, tag="pr_sb")
                    nc.vector.tensor_copy(pr_sb, pr_ps)
                    nc.sync.dma_start(
                        out=out_d[(qb * 4 + s) * 128:(qb * 4 + s + 1) * 128,
                                  n * TB:(n + 1) * TB],
                        in_=pr_sb,
                    )


def shard_inputs(x, w_qkv, w_proj):
    """Full inputs -> list of 8 per-core input maps."""
    cosT, sinT, prot, mask01 = _host_constants()
    x = np.ascontiguousarray(np.asarray(x, dtype=np.float32))
    w_qkv = np.asarray(w_qkv, dtype=np.float32)
    w_proj = np.asarray(w_proj, dtype=np.float32)
    in_maps = []
    for c in range(N_CORES):
        b, g = c // TPG, c % TPG
        xT = np.ascontiguousarray(x[b].T)                     # [DIM, T]
        wq = w_qkv[:, g * LOC:(g + 1) * LOC]
        wk = w_qkv[:, INNER + g * LOC:INNER + (g + 1) * LOC]
        wv = w_qkv[:, 2 * INNER + g * LOC:2 * INNER + (g + 1) * LOC]
        w_qk = np.ascontiguousarray(np.concatenate([wq, wk], axis=1))  # [DIM, 512]
        w_pr = np.ascontiguousarray(w_proj[g * LOC:(g + 1) * LOC, :])  # [256, DIM]
        in_maps.append({
            "xT": xT.astype(ml_dtypes.bfloat16),
            "w_qk": w_qk.astype(ml_dtypes.bfloat16),
            "w_v": np.ascontiguousarray(wv).astype(ml_dtypes.bfloat16),
            "w_pr": w_pr.astype(ml_dtypes.bfloat16),
            "cosT": cosT,
            "sinT": sinT,
            "prot": prot.astype(ml_dtypes.bfloat16),
            "mask01": mask01,
        })
    return in_maps


_CACHE = {}


def _get_compiled():
    if "nc" not in _CACHE:
        nc = bacc.Bacc("TRN2", target_bir_lowering=False, debug=False,
                       enable_asserts=True, num_devices=N_CORES)
        with tile.TileContext(nc) as tc:
            build_kernel(tc)
        nc.compile()
        _CACHE["nc"] = nc
    return _CACHE["nc"]


def kernel(x, w_qkv, w_proj):
    nc = _get_compiled()
    in_maps = shard_inputs(x, w_qkv, w_proj)
    res = run_bass_kernel_spmd(nc, in_maps, core_ids=list(range(N_CORES)))
    outs = [res.results[c]["out"] for c in range(N_CORES)]
    full = np.stack([
        np.sum([outs[b * TPG + g] for g in range(TPG)], axis=0, dtype=np.float32)
        for b in range(B)
    ])
    return full.astype(np.float32)


# revision 12
# speedup vs baseline: 1.5940x; 1.0304x over previous
"""Causal self-attention with RoPE on 8 TRN2 NeuronCores.

Sharding: 2 (batch) x 4 (head-group tensor parallel). Core c handles
batch b=c//4 and heads [4g, 4g+4) with g=c%4. Each core computes its
q,k,v projections, RoPE, causal attention (transposed-scores flash
layout), and its partial of the output projection; the host sums the
4 partials per batch (the "all-reduce").

v2: fused per-t-block loop (QKV(tb) -> attention(qb=tb) -> proj(qb))
so PE stays dense and ACT/DVE overlap; pair-split double-buffered
score chunks; batched approximate reciprocal for softmax norms.

Self-contained: hardcodes shapes from the problem spec.
"""
import numpy as np
import ml_dtypes

import concourse.bass as bass
import concourse.mybir as mybir
import concourse.tile as tile
from concourse import bacc
from concourse.bass_utils import run_bass_kernel_spmd

F32 = mybir.dt.float32
BF16 = mybir.dt.bfloat16

B, T, DIM = 2, 2048, 1024
HEADS, HEAD_DIM = 16, 64
INNER = HEADS * HEAD_DIM
ROPE_BASE = 10000.0
N_CORES = 8
TPG = 4                      # tensor-parallel group size (head groups)
HPC = HEADS // TPG           # heads per core = 4
LOC = HPC * HEAD_DIM         # local inner = 256
SCALE = 1.0 / np.sqrt(HEAD_DIM)

TB = 512                     # t block for QKV / q block for attention
NTB = T // TB                # 4
NKT = T // 128               # 16 k tiles
ND = DIM // 128              # 8 contraction chunks


def _host_constants():
    inv_freq = 1.0 / (ROPE_BASE ** (np.arange(0, HEAD_DIM, 2, dtype=np.float32) / HEAD_DIM))
    t = np.arange(T, dtype=np.float32)
    freqs = np.outer(t, inv_freq).astype(np.float32)          # [T, 32]
    cos32 = np.cos(freqs).T.astype(np.float32)                # [32, T]
    sin32 = np.sin(freqs).T.astype(np.float32)
    cosT = np.tile(cos32, (4, 1))                             # [128, T]
    sinT = np.tile(sin32, (4, 1))

    # rot matrix: rot[m] = -x[m+32] (m%64<32), +x[m-32] (m%64>=32); lhsT[k, m]
    prot = np.zeros((128, 128), dtype=np.float32)
    for blk in range(2):
        o = blk * 64
        for m in range(32):
            prot[o + m + 32, o + m] = -1.0
            prot[o + m, o + m + 32] = 1.0

    # post-exp 0/1 causal mask for the diagonal 128-col block: keep j >= p
    j = np.arange(128)[None, :]
    p = np.arange(128)[:, None]
    mask01 = (j >= p).astype(ml_dtypes.bfloat16)              # [128, 128]
    return cosT, sinT, prot, mask01


def build_kernel(tc):
    nc = tc.nc
    xT = nc.dram_tensor("xT", [DIM, T], BF16, kind="ExternalInput").ap()
    w_qk = nc.dram_tensor("w_qk", [DIM, 2 * LOC], BF16, kind="ExternalInput").ap()
    w_v = nc.dram_tensor("w_v", [DIM, LOC], BF16, kind="ExternalInput").ap()
    w_pr = nc.dram_tensor("w_pr", [LOC, DIM], BF16, kind="ExternalInput").ap()
    cosT_d = nc.dram_tensor("cosT", [128, T], F32, kind="ExternalInput").ap()
    sinT_d = nc.dram_tensor("sinT", [128, T], F32, kind="ExternalInput").ap()
    prot_d = nc.dram_tensor("prot", [128, 128], BF16, kind="ExternalInput").ap()
    mask01_d = nc.dram_tensor("mask01", [128, 128], BF16, kind="ExternalInput").ap()
    out_d = nc.dram_tensor("out", [T, DIM], F32, kind="ExternalOutput").ap()

    with (
        tc.tile_pool(name="const", bufs=1) as const,
        tc.tile_pool(name="xt", bufs=2) as xt_pool,
        tc.tile_pool(name="persist", bufs=1) as persist,
        tc.tile_pool(name="work", bufs=4) as work,
        tc.tile_pool(name="expp", bufs=6) as expp,
        tc.tile_pool(name="ps_sc", bufs=2, space="PSUM") as ps_sc,
        tc.tile_pool(name="ps_acc", bufs=1, space="PSUM") as ps_acc,
        tc.tile_pool(name="ps_mm", bufs=1, space="PSUM") as ps_mm,
        tc.tile_pool(name="dram", bufs=2, space="DRAM") as dram,
    ):
        # ---- tb=0 activations + qk weights first: PE starts ASAP ----
        xt_sb = {}
        wqk_sb = []
        for d in range(ND):
            x_t = xt_pool.tile([128, TB], BF16, tag=f"xt{d}", name=f"xt{d}_0")
            nc.sync.dma_start(out=x_t, in_=xT[d * 128:(d + 1) * 128, 0:TB])
            xt_sb[(0, d)] = x_t
            wq_t = const.tile([128, 2 * LOC], BF16, tag=f"wqk{d}", name=f"wqk{d}")
            nc.sync.dma_start(out=wq_t, in_=w_qk[d * 128:(d + 1) * 128, :])
            wqk_sb.append(wq_t)
        prot_sb = const.tile([128, 128], BF16, tag="prot")
        nc.sync.dma_start(out=prot_sb, in_=prot_d)
        cos_sb = const.tile([128, T], F32, tag="cos")
        nc.sync.dma_start(out=cos_sb, in_=cosT_d)
        sin_sb = const.tile([128, T], F32, tag="sin")
        nc.sync.dma_start(out=sin_sb, in_=sinT_d)
        wv_sb = []
        for d in range(ND):
            wv_t = const.tile([128, LOC], BF16, tag=f"wv{d}", name=f"wv{d}")
            nc.sync.dma_start(out=wv_t, in_=w_v[d * 128:(d + 1) * 128, :])
            wv_sb.append(wv_t)
        mask_sb = const.tile([128, 128], BF16, tag="mask")
        nc.sync.dma_start(out=mask_sb, in_=mask01_d)
        ones_sb = const.tile([128, 1], BF16, tag="ones")
        nc.vector.memset(ones_sb, 1.0)
        ones2_sb = const.tile([128, 64], F32, tag="ones2")
        nc.vector.memset(ones2_sb, 1.0)
        wpr_sb = []
        for c in range(2):
            wp_t = const.tile([128, DIM], BF16, tag=f"wpr{c}", name=f"wpr{c}")
            nc.sync.dma_start(out=wp_t, in_=w_pr[c * 128:(c + 1) * 128, :])
            wpr_sb.append(wp_t)

        # persistent per-phase outputs
        qk_rope = [[persist.tile([128, TB], BF16, tag=f"qkr{m}_{tb}", name=f"qkr{m}_{tb}")
                    for tb in range(NTB)] for m in range(4)]
        v_sb = [persist.tile([128, LOC], BF16, tag=f"v{ts}", name=f"v{ts}")
                for ts in range(NKT)]

        mask_bc = mask_sb.rearrange("p (o n) -> p o n", o=1).to_broadcast([128, 2, 128])

        for tb in range(NTB):
            # prefetch next tb's activations (bufs=2 -> depth-1 prefetch)
            if tb > 0:
                for d in range(ND):
                    x_t = xt_pool.tile([128, TB], BF16, tag=f"xt{d}", name=f"xt{d}_{tb}")
                    nc.sync.dma_start(
                        out=x_t, in_=xT[d * 128:(d + 1) * 128, tb * TB:(tb + 1) * TB])
                    xt_sb[(tb, d)] = x_t

            # ---- QKV + RoPE for this t block ----
            for m in range(4):  # m 0,1 -> q pairs; 2,3 -> k pairs
                qk1 = (ps_mm.tile([128, TB], F32, tag="mm", name=f"qk1_{m}_{tb}")
                       if m % 2 == 0 else
                       ps_acc.tile([128, TB], F32, tag="av1", name=f"qk1_{m}_{tb}"))
                for d in range(ND):
                    nc.tensor.matmul(
                        qk1,
                        lhsT=wqk_sb[d][:, m * 128:(m + 1) * 128],
                        rhs=xt_sb[(tb, d)],
                        start=(d == 0), stop=(d == ND - 1),
                    )
                raw_sb = work.tile([128, TB], BF16, tag="raw")
                nc.scalar.copy(raw_sb, qk1)
                rot_ps = ps_acc.tile([128, TB], F32, tag="av0", name=f"rot_{m}_{tb}")
                nc.tensor.matmul(rot_ps, lhsT=prot_sb, rhs=raw_sb,
                                 start=True, stop=True)
                qc_sb = work.tile([128, TB], F32, tag="qc")
                nc.vector.tensor_mul(qc_sb, raw_sb, cos_sb[:, tb * TB:(tb + 1) * TB])
                rs_sb = work.tile([128, TB], F32, tag="rs")
                nc.vector.tensor_mul(rs_sb, rot_ps, sin_sb[:, tb * TB:(tb + 1) * TB])
                nc.vector.tensor_add(qk_rope[m][tb], qc_sb, rs_sb)
            for s in range(4):
                ts = tb * 4 + s
                v_ps = ps_mm.tile([128, LOC], F32, tag="mm", name=f"v_ps{ts}")
                for d in range(ND):
                    nc.tensor.matmul(
                        v_ps,
                        lhsT=xt_sb[(tb, d)][:, s * 128:(s + 1) * 128],
                        rhs=wv_sb[d],
                        start=(d == 0), stop=(d == ND - 1),
                    )
                nc.scalar.copy(v_sb[ts], v_ps)

            # ---- attention for qb=tb (scoresT layout, flash over kt) ----
            qb = tb
            nkt = 4 * (qb + 1)
            av_ps = [ps_acc.tile([128, TB], F32, tag=f"av{p}", name=f"av{p}_{qb}")
                     for p in range(2)]
            rsum_ps = ps_acc.tile([128, TB], F32, tag="rsum", name=f"rsum_{qb}")
            for kt in range(nkt):
                ktl = kt - 4 * qb
                a = 128 * ktl if ktl >= 0 else 0
                w = TB - a
                tbk, ok = kt // 4, (kt % 4) * 128
                for p in range(2):
                    sc2 = ps_sc.tile([128, 2, TB], F32, tag="sc", name=f"sc{qb}_{kt}_{p}")
                    for j in range(2):
                        nc.tensor.matmul(
                            sc2[:, j, 0:w],
                            lhsT=qk_rope[2 + p][tbk][64 * j:64 * j + 64, ok:ok + 128],
                            rhs=qk_rope[p][qb][64 * j:64 * j + 64, a:TB],
                            start=True, stop=True, tile_position=(64 * j, 0),
                        )
                    exp2 = expp.tile([128, 2, TB], BF16, tag="exp", name=f"exp{qb}_{kt}_{p}")
                    nc.scalar.activation(exp2[:, :, 0:w], sc2[:, :, 0:w],
                                         mybir.ActivationFunctionType.Exp,
                                         scale=float(SCALE))
                    if ktl >= 0:
                        nc.vector.tensor_mul(exp2[:, :, 0:128], exp2[:, :, 0:128],
                                             mask_bc)
                    for j in range(2):
                        h = 2 * p + j
                        nc.tensor.matmul(
                            av_ps[p][64 * j:64 * j + 64, a:TB],
                            lhsT=v_sb[kt][:, 64 * h:64 * h + 64],
                            rhs=exp2[:, j, 0:w],
                            start=(kt == 0), stop=(kt == nkt - 1),
                            skip_group_check=True,
                            tile_position=(0, 64 * j),
                        )
                    for j in range(2):
                        h = 2 * p + j
                        nc.tensor.matmul(
                            rsum_ps[32 * h:32 * h + 1, a:TB],
                            lhsT=ones_sb,
                            rhs=exp2[:, j, 0:w],
                            start=(kt == 0), stop=(kt == nkt - 1),
                            skip_group_check=True,
                            tile_position=(0, 32 * h),
                        )

            # softmax normalizer: copy rowsums to SBUF, replicate via K=1
            # outer-product matmuls, then approx reciprocal on the dense tile
            rsum_sb = work.tile([128, TB], F32, tag="recip")
            nc.vector.tensor_copy(rsum_sb, rsum_ps)
            bc2 = ps_sc.tile([128, 2, TB], F32, tag="sc", name=f"bc{qb}")
            for p in range(2):
                for j in range(2):
                    h = 2 * p + j
                    nc.tensor.matmul(
                        bc2[64 * j:64 * j + 64, p, :],
                        lhsT=ones2_sb[32 * h:32 * h + 1, :],
                        rhs=rsum_sb[32 * h:32 * h + 1, :],
                        start=True, stop=True, skip_group_check=True,
                        tile_position=(32 * h, 64 * j),
                    )
            recip2_sb = work.tile([128, 2, TB], F32, tag="recipb")
            scr_sb = work.tile([128, 2, TB], F32, tag="rscr")
            nc.vector.reciprocal_approx_accurate(
                out=recip2_sb, in_=bc2, scratch=scr_sb)
            outT_sb = []
            for p in range(2):
                o_t = persist.tile([128, TB], BF16, tag=f"outT{qb}_{p}", name=f"outT{qb}_{p}")
                nc.vector.tensor_mul(o_t, av_ps[p], recip2_sb[:, p, :])
                outT_sb.append(o_t)

            # ---- output projection for this qb's 4 t-subtiles ----
            for s in range(4):
                for n in range(2):
                    pr2 = ps_sc.tile([128, 2, TB], F32, tag="sc", name=f"pr{qb}_{s}_{n}")
                    pr_ps = pr2[:, 0, :]
                    for p in range(2):
                        nc.tensor.matmul(
                            pr_ps,
                            lhsT=outT_sb[p][:, s * 128:(s + 1) * 128],
                            rhs=wpr_sb[p][:, n * TB:(n + 1) * TB],
                            start=(p == 0), stop=(p == 1),
                        )
                    pr_sb = work.tile([128, TB], F32, tag="pr_sb")
                    nc.vector.tensor_copy(pr_sb, pr_ps)
                    nc.sync.dma_start(
                        out=out_d[(qb * 4 + s) * 128:(qb * 4 + s + 1) * 128,
                                  n * TB:(n + 1) * TB],
                        in_=pr_sb,
                    )


def shard_inputs(x, w_qkv, w_proj):
    """Full inputs -> list of 8 per-core input maps."""
    cosT, sinT, prot, mask01 = _host_constants()
    x = np.ascontiguousarray(np.asarray(x, dtype=np.float32))
    w_qkv = np.asarray(w_qkv, dtype=np.float32)
    w_proj = np.asarray(w_proj, dtype=np.float32)
    in_maps = []
    for c in range(N_CORES):
        b, g = c // TPG, c % TPG
        xT = np.ascontiguousarray(x[b].T)                     # [DIM, T]
        wq = w_qkv[:, g * LOC:(g + 1) * LOC]
        wk = w_qkv[:, INNER + g * LOC:INNER + (g + 1) * LOC]
        wv = w_qkv[:, 2 * INNER + g * LOC:2 * INNER + (g + 1) * LOC]
        w_qk = np.ascontiguousarray(np.concatenate([wq, wk], axis=1))  # [DIM, 512]
        w_pr = np.ascontiguousarray(w_proj[g * LOC:(g + 1) * LOC, :])  # [256, DIM]
        in_maps.append({
            "xT": xT.astype(ml_dtypes.bfloat16),
            "w_qk": w_qk.astype(ml_dtypes.bfloat16),
            "w_v": np.ascontiguousarray(wv).astype(ml_dtypes.bfloat16),
            "w_pr": w_pr.astype(ml_dtypes.bfloat16),
            "cosT": cosT,
            "sinT": sinT,
            "prot": prot.astype(ml_dtypes.bfloat16),
            "mask01": mask01,
        })
    return in_maps


_CACHE = {}


def _get_compiled():
    if "nc" not in _CACHE:
        nc = bacc.Bacc("TRN2", target_bir_lowering=False, debug=False,
                       enable_asserts=True, num_devices=N_CORES)
        with tile.TileContext(nc) as tc:
            build_kernel(tc)
        nc.compile()
        _CACHE["nc"] = nc
    return _CACHE["nc"]


def kernel(x, w_qkv, w_proj):
    nc = _get_compiled()
    in_maps = shard_inputs(x, w_qkv, w_proj)
    res = run_bass_kernel_spmd(nc, in_maps, core_ids=list(range(N_CORES)))
    outs = [res.results[c]["out"] for c in range(N_CORES)]
    full = np.stack([
        np.sum([outs[b * TPG + g] for g in range(TPG)], axis=0, dtype=np.float32)
        for b in range(B)
    ])
    return full.astype(np.float32)


# revision 15
# speedup vs baseline: 1.6498x; 1.0350x over previous
"""Causal self-attention with RoPE on 8 TRN2 NeuronCores.

Sharding: 2 (batch) x 4 (head-group tensor parallel). Core c handles
batch b=c//4 and heads [4g, 4g+4) with g=c%4. Each core computes its
q,k,v projections, RoPE, causal attention (transposed-scores flash
layout), and its partial of the output projection; the host sums the
4 partials per batch (the "all-reduce").

v2: fused per-t-block loop (QKV(tb) -> attention(qb=tb) -> proj(qb))
so PE stays dense and ACT/DVE overlap; pair-split double-buffered
score chunks; batched approximate reciprocal for softmax norms.

Self-contained: hardcodes shapes from the problem spec.
"""
import numpy as np
import ml_dtypes

import concourse.bass as bass
import concourse.mybir as mybir
import concourse.tile as tile
from concourse import bacc
from concourse.bass_utils import run_bass_kernel_spmd

F32 = mybir.dt.float32
BF16 = mybir.dt.bfloat16

B, T, DIM = 2, 2048, 1024
HEADS, HEAD_DIM = 16, 64
INNER = HEADS * HEAD_DIM
ROPE_BASE = 10000.0
N_CORES = 8
TPG = 4                      # tensor-parallel group size (head groups)
HPC = HEADS // TPG           # heads per core = 4
LOC = HPC * HEAD_DIM         # local inner = 256
SCALE = 1.0 / np.sqrt(HEAD_DIM)

TB = 512                     # t block for QKV / q block for attention
NTB = T // TB                # 4
NKT = T // 128               # 16 k tiles
ND = DIM // 128              # 8 contraction chunks


def _host_constants():
    inv_freq = 1.0 / (ROPE_BASE ** (np.arange(0, HEAD_DIM, 2, dtype=np.float32) / HEAD_DIM))
    t = np.arange(T, dtype=np.float32)
    freqs = np.outer(t, inv_freq).astype(np.float32)          # [T, 32]
    cos32 = np.cos(freqs).T.astype(np.float32)                # [32, T]
    sin32 = np.sin(freqs).T.astype(np.float32)
    cosT = np.tile(cos32, (4, 1))                             # [128, T]
    sinT = np.tile(sin32, (4, 1))

    # rot matrix: rot[m] = -x[m+32] (m%64<32), +x[m-32] (m%64>=32); lhsT[k, m]
    prot = np.zeros((128, 128), dtype=np.float32)
    for blk in range(2):
        o = blk * 64
        for m in range(32):
            prot[o + m + 32, o + m] = -1.0
            prot[o + m, o + m + 32] = 1.0

    # post-exp 0/1 causal mask for the diagonal 128-col block: keep j >= p
    j = np.arange(128)[None, :]
    p = np.arange(128)[:, None]
    mask01 = (j >= p).astype(ml_dtypes.bfloat16)              # [128, 128]
    return cosT, sinT, prot, mask01


def build_kernel(tc):
    nc = tc.nc
    xT = nc.dram_tensor("xT", [DIM, T], BF16, kind="ExternalInput").ap()
    w_qk = nc.dram_tensor("w_qk", [DIM, 2 * LOC], BF16, kind="ExternalInput").ap()
    w_v = nc.dram_tensor("w_v", [DIM, LOC], BF16, kind="ExternalInput").ap()
    w_pr = nc.dram_tensor("w_pr", [LOC, DIM], BF16, kind="ExternalInput").ap()
    cosT_d = nc.dram_tensor("cosT", [128, T], F32, kind="ExternalInput").ap()
    sinT_d = nc.dram_tensor("sinT", [128, T], F32, kind="ExternalInput").ap()
    prot_d = nc.dram_tensor("prot", [128, 128], BF16, kind="ExternalInput").ap()
    mask01_d = nc.dram_tensor("mask01", [128, 128], BF16, kind="ExternalInput").ap()
    out_d = nc.dram_tensor("out", [T, DIM], BF16, kind="ExternalOutput").ap()

    with (
        tc.tile_pool(name="const", bufs=1) as const,
        tc.tile_pool(name="xt", bufs=2) as xt_pool,
        tc.tile_pool(name="persist", bufs=1) as persist,
        tc.tile_pool(name="work", bufs=4) as work,
        tc.tile_pool(name="expp", bufs=6) as expp,
        tc.tile_pool(name="ps_sc", bufs=2, space="PSUM") as ps_sc,
        tc.tile_pool(name="ps_acc", bufs=1, space="PSUM") as ps_acc,
        tc.tile_pool(name="ps_mm", bufs=1, space="PSUM") as ps_mm,
        tc.tile_pool(name="dram", bufs=2, space="DRAM") as dram,
    ):
        # ---- tb=0 activations + qk weights first: PE starts ASAP ----
        xt_sb = {}
        wqk_sb = []
        for d in range(ND):
            x_t = xt_pool.tile([128, TB], BF16, tag=f"xt{d}", name=f"xt{d}_0")
            nc.sync.dma_start(out=x_t, in_=xT[d * 128:(d + 1) * 128, 0:TB])
            xt_sb[(0, d)] = x_t
            wq_t = const.tile([128, 2 * LOC], BF16, tag=f"wqk{d}", name=f"wqk{d}")
            nc.sync.dma_start(out=wq_t, in_=w_qk[d * 128:(d + 1) * 128, :])
            wqk_sb.append(wq_t)
        prot_sb = const.tile([128, 128], BF16, tag="prot")
        nc.sync.dma_start(out=prot_sb, in_=prot_d)
        cos_sb = const.tile([128, T], F32, tag="cos")
        nc.sync.dma_start(out=cos_sb, in_=cosT_d)
        sin_sb = const.tile([128, T], F32, tag="sin")
        nc.sync.dma_start(out=sin_sb, in_=sinT_d)
        wv_sb = []
        for d in range(ND):
            wv_t = const.tile([128, LOC], BF16, tag=f"wv{d}", name=f"wv{d}")
            nc.sync.dma_start(out=wv_t, in_=w_v[d * 128:(d + 1) * 128, :])
            wv_sb.append(wv_t)
        mask_sb = const.tile([128, 128], BF16, tag="mask")
        nc.sync.dma_start(out=mask_sb, in_=mask01_d)
        ones_sb = const.tile([128, 1], BF16, tag="ones")
        nc.vector.memset(ones_sb, 1.0)
        ones2_sb = const.tile([128, 64], F32, tag="ones2")
        nc.vector.memset(ones2_sb, 1.0)
        wpr_sb = []
        for c in range(2):
            wp_t = const.tile([128, DIM], BF16, tag=f"wpr{c}", name=f"wpr{c}")
            nc.sync.dma_start(out=wp_t, in_=w_pr[c * 128:(c + 1) * 128, :])
            wpr_sb.append(wp_t)

        # persistent per-phase outputs
        qk_rope = [[persist.tile([128, TB], BF16, tag=f"qkr{m}_{tb}", name=f"qkr{m}_{tb}")
                    for tb in range(NTB)] for m in range(4)]
        v_sb = [persist.tile([128, LOC], BF16, tag=f"v{ts}", name=f"v{ts}")
                for ts in range(NKT)]

        mask_bc = mask_sb.rearrange("p (o n) -> p o n", o=1).to_broadcast([128, 2, 128])

        hoisted = {}
        for tb in range(NTB):
            # ---- QKV + RoPE for this t block ----
            for m in range(4):  # m 0,1 -> q pairs; 2,3 -> k pairs
                if m == 0 and tb in hoisted:
                    raw_sb = hoisted.pop(tb)
                else:
                    qk1 = (ps_mm.tile([128, TB], F32, tag="mm", name=f"qk1_{m}_{tb}")
                           if m % 2 == 0 else
                           ps_acc.tile([128, TB], F32, tag="av1", name=f"qk1_{m}_{tb}"))
                    for d in range(ND):
                        nc.tensor.matmul(
                            qk1,
                            lhsT=wqk_sb[d][:, m * 128:(m + 1) * 128],
                            rhs=xt_sb[(tb, d)],
                            start=(d == 0), stop=(d == ND - 1),
                        )
                    raw_sb = work.tile([128, TB], BF16, tag="raw")
                    nc.scalar.copy(raw_sb, qk1)
                rot_ps = ps_acc.tile([128, TB], F32, tag="av0", name=f"rot_{m}_{tb}")
                nc.tensor.matmul(rot_ps, lhsT=prot_sb, rhs=raw_sb,
                                 start=True, stop=True)
                qc_sb = work.tile([128, TB], F32, tag="qc")
                nc.vector.tensor_mul(qc_sb, raw_sb, cos_sb[:, tb * TB:(tb + 1) * TB])
                rs_sb = work.tile([128, TB], F32, tag="rs")
                nc.vector.tensor_mul(rs_sb, rot_ps, sin_sb[:, tb * TB:(tb + 1) * TB])
                nc.vector.tensor_add(qk_rope[m][tb], qc_sb, rs_sb)
            for s in range(4):
                ts = tb * 4 + s
                v_ps = ps_mm.tile([128, LOC], F32, tag="mm", name=f"v_ps{ts}")
                for d in range(ND):
                    nc.tensor.matmul(
                        v_ps,
                        lhsT=xt_sb[(tb, d)][:, s * 128:(s + 1) * 128],
                        rhs=wv_sb[d],
                        start=(d == 0), stop=(d == ND - 1),
                    )
                nc.scalar.copy(v_sb[ts], v_ps)

            # ---- attention for qb=tb (scoresT layout, flash over kt) ----
            qb = tb
            nkt = 4 * (qb + 1)
            av_ps = [ps_acc.tile([128, TB], F32, tag=f"av{p}", name=f"av{p}_{qb}")
                     for p in range(2)]
            rsum_ps = ps_acc.tile([128, TB], F32, tag="rsum", name=f"rsum_{qb}")
            for kt in range(nkt):
                ktl = kt - 4 * qb
                a = 128 * ktl if ktl >= 0 else 0
                w = TB - a
                tbk, ok = kt // 4, (kt % 4) * 128
                for p in range(2):
                    sc2 = ps_sc.tile([128, 2, TB], F32, tag="sc", name=f"sc{qb}_{kt}_{p}")
                    for j in range(2):
                        nc.tensor.matmul(
                            sc2[:, j, 0:w],
                            lhsT=qk_rope[2 + p][tbk][64 * j:64 * j + 64, ok:ok + 128],
                            rhs=qk_rope[p][qb][64 * j:64 * j + 64, a:TB],
                            start=True, stop=True, tile_position=(64 * j, 0),
                        )
                    exp2 = expp.tile([128, 2, TB], BF16, tag="exp", name=f"exp{qb}_{kt}_{p}")
                    nc.scalar.activation(exp2[:, :, 0:w], sc2[:, :, 0:w],
                                         mybir.ActivationFunctionType.Exp,
                                         scale=float(SCALE))
                    if ktl >= 0:
                        nc.vector.tensor_mul(exp2[:, :, 0:128], exp2[:, :, 0:128],
                                             mask_bc)
                    for j in range(2):
                        h = 2 * p + j
                        nc.tensor.matmul(
                            av_ps[p][64 * j:64 * j + 64, a:TB],
                            lhsT=v_sb[kt][:, 64 * h:64 * h + 64],
                            rhs=exp2[:, j, 0:w],
                            start=(kt == 0), stop=(kt == nkt - 1),
                            skip_group_check=True,
                            tile_position=(0, 64 * j),
                        )
                    for j in range(2):
                        h = 2 * p + j
                        nc.tensor.matmul(
                            rsum_ps[32 * h:32 * h + 1, a:TB],
                            lhsT=ones_sb,
                            rhs=exp2[:, j, 0:w],
                            start=(kt == 0), stop=(kt == nkt - 1),
                            skip_group_check=True,
                            tile_position=(0, 32 * h),
                        )

            # hoist next window's first q projection so the PE has work
            # during the normalizer's DVE latency
            if tb + 1 < NTB:
                for d in range(ND):
                    x_t = xt_pool.tile([128, TB], BF16, tag=f"xt{d}",
                                       name=f"xt{d}_{tb + 1}")
                    nc.sync.dma_start(
                        out=x_t,
                        in_=xT[d * 128:(d + 1) * 128, (tb + 1) * TB:(tb + 2) * TB])
                    xt_sb[(tb + 1, d)] = x_t
                qk1h = ps_mm.tile([128, TB], F32, tag="mm", name=f"qk1_0_{tb + 1}")
                for d in range(ND):
                    nc.tensor.matmul(
                        qk1h,
                        lhsT=wqk_sb[d][:, 0:128],
                        rhs=xt_sb[(tb + 1, d)],
                        start=(d == 0), stop=(d == ND - 1),
                    )
                rawh = work.tile([128, TB], BF16, tag="raw")
                nc.scalar.copy(rawh, qk1h)
                hoisted[tb + 1] = rawh

            # softmax normalizer: copy rowsums to SBUF, replicate via K=1
            # outer-product matmuls, then approx reciprocal on the dense tile
            rsum_sb = work.tile([128, TB], F32, tag="recip")
            nc.vector.tensor_copy(rsum_sb, rsum_ps)
            bc2 = ps_sc.tile([128, 2, TB], F32, tag="sc", name=f"bc{qb}")
            for p in range(2):
                for j in range(2):
                    h = 2 * p + j
                    nc.tensor.matmul(
                        bc2[64 * j:64 * j + 64, p, :],
                        lhsT=ones2_sb[32 * h:32 * h + 1, :],
                        rhs=rsum_sb[32 * h:32 * h + 1, :],
                        start=True, stop=True, skip_group_check=True,
                        tile_position=(32 * h, 64 * j),
                    )
            recip2_sb = work.tile([128, 2, TB], F32, tag="recipb")
            nc.vector.reciprocal_approx_fast(out=recip2_sb, in_=bc2)
            outT_sb = []
            for p in range(2):
                o_t = persist.tile([128, TB], BF16, tag=f"outT{qb}_{p}", name=f"outT{qb}_{p}")
                nc.vector.tensor_mul(o_t, av_ps[p], recip2_sb[:, p, :])
                outT_sb.append(o_t)

            # ---- output projection for this qb's 4 t-subtiles ----
            for s in range(4):
                for n in range(2):
                    pr2 = ps_sc.tile([128, 2, TB], F32, tag="sc", name=f"pr{qb}_{s}_{n}")
                    pr_ps = pr2[:, 0, :]
                    for p in range(2):
                        nc.tensor.matmul(
                            pr_ps,
                            lhsT=outT_sb[p][:, s * 128:(s + 1) * 128],
                            rhs=wpr_sb[p][:, n * TB:(n + 1) * TB],
                            start=(p == 0), stop=(p == 1),
                        )
                    pr_sb = work.tile([128, TB], BF16, tag="pr_sb")
                    nc.vector.tensor_copy(pr_sb, pr_ps)
                    nc.sync.dma_start(
                        out=out_d[(qb * 4 + s) * 128:(qb * 4 + s + 1) * 128,
                                  n * TB:(n + 1) * TB],
                        in_=pr_sb,
                    )


def shard_inputs(x, w_qkv, w_proj):
    """Full inputs -> list of 8 per-core input maps."""
    cosT, sinT, prot, mask01 = _host_constants()
    x = np.ascontiguousarray(np.asarray(x, dtype=np.float32))
    w_qkv = np.asarray(w_qkv, dtype=np.float32)
    w_proj = np.asarray(w_proj, dtype=np.float32)
    in_maps = []
    for c in range(N_CORES):
        b, g = c // TPG, c % TPG
        xT = np.ascontiguousarray(x[b].T)                     # [DIM, T]
        wq = w_qkv[:, g * LOC:(g + 1) * LOC]
        wk = w_qkv[:, INNER + g * LOC:INNER + (g + 1) * LOC]
        wv = w_qkv[:, 2 * INNER + g * LOC:2 * INNER + (g + 1) * LOC]
        w_qk = np.ascontiguousarray(np.concatenate([wq, wk], axis=1))  # [DIM, 512]
        w_pr = np.ascontiguousarray(w_proj[g * LOC:(g + 1) * LOC, :])  # [256, DIM]
        in_maps.append({
            "xT": xT.astype(ml_dtypes.bfloat16),
            "w_qk": w_qk.astype(ml_dtypes.bfloat16),
            "w_v": np.ascontiguousarray(wv).astype(ml_dtypes.bfloat16),
            "w_pr": w_pr.astype(ml_dtypes.bfloat16),
            "cosT": cosT,
            "sinT": sinT,
            "prot": prot.astype(ml_dtypes.bfloat16),
            "mask01": mask01,
        })
    return in_maps


_CACHE = {}


def _get_compiled():
    if "nc" not in _CACHE:
        nc = bacc.Bacc("TRN2", target_bir_lowering=False, debug=False,
                       enable_asserts=True, num_devices=N_CORES)
        with tile.TileContext(nc) as tc:
            build_kernel(tc)
        nc.compile()
        _CACHE["nc"] = nc
    return _CACHE["nc"]


def kernel(x, w_qkv, w_proj):
    nc = _get_compiled()
    in_maps = shard_inputs(x, w_qkv, w_proj)
    res = run_bass_kernel_spmd(nc, in_maps, core_ids=list(range(N_CORES)))
    outs = [res.results[c]["out"] for c in range(N_CORES)]
    full = np.stack([
        np.sum([outs[b * TPG + g] for g in range(TPG)], axis=0, dtype=np.float32)
        for b in range(B)
    ])
    return full.astype(np.float32)


# revision 16
# speedup vs baseline: 1.8546x; 1.1241x over previous
"""Causal self-attention with RoPE on 8 TRN2 NeuronCores.

Sharding: 2 (batch) x 4 (head-group tensor parallel). Core c handles
batch b=c//4 and heads [4g, 4g+4) with g=c%4. Each core computes its
q,k,v projections, RoPE, causal attention (transposed-scores flash
layout), and its partial of the output projection; the host sums the
4 partials per batch (the "all-reduce").

v2: fused per-t-block loop (QKV(tb) -> attention(qb=tb) -> proj(qb))
so PE stays dense and ACT/DVE overlap; pair-split double-buffered
score chunks; batched approximate reciprocal for softmax norms.

Self-contained: hardcodes shapes from the problem spec.
"""
import numpy as np
import ml_dtypes

import concourse.bass as bass
import concourse.mybir as mybir
import concourse.tile as tile
from concourse import bacc
from concourse.bass_utils import run_bass_kernel_spmd

F32 = mybir.dt.float32
BF16 = mybir.dt.bfloat16

B, T, DIM = 2, 2048, 1024
HEADS, HEAD_DIM = 16, 64
INNER = HEADS * HEAD_DIM
ROPE_BASE = 10000.0
N_CORES = 8
TPG = 4                      # tensor-parallel group size (head groups)
HPC = HEADS // TPG           # heads per core = 4
LOC = HPC * HEAD_DIM         # local inner = 256
SCALE = 1.0 / np.sqrt(HEAD_DIM)

TB = 512                     # t block for QKV / q block for attention
NTB = T // TB                # 4
NKT = T // 128               # 16 k tiles
ND = DIM // 128              # 8 contraction chunks


def _host_constants():
    inv_freq = 1.0 / (ROPE_BASE ** (np.arange(0, HEAD_DIM, 2, dtype=np.float32) / HEAD_DIM))
    t = np.arange(T, dtype=np.float32)
    freqs = np.outer(t, inv_freq).astype(np.float32)          # [T, 32]
    cos32 = np.cos(freqs).T.astype(np.float32)                # [32, T]
    sin32 = np.sin(freqs).T.astype(np.float32)
    cosT = np.tile(cos32, (4, 1))                             # [128, T]
    sinT = np.tile(sin32, (4, 1))

    # rot matrix: rot[m] = -x[m+32] (m%64<32), +x[m-32] (m%64>=32); lhsT[k, m]
    prot = np.zeros((128, 128), dtype=np.float32)
    for blk in range(2):
        o = blk * 64
        for m in range(32):
            prot[o + m + 32, o + m] = -1.0
            prot[o + m, o + m + 32] = 1.0

    # post-exp 0/1 causal mask for the diagonal 128-col block: keep j >= p
    j = np.arange(128)[None, :]
    p = np.arange(128)[:, None]
    mask01 = (j >= p).astype(ml_dtypes.bfloat16)              # [128, 128]
    return cosT, sinT, prot, mask01


def build_kernel(tc):
    nc = tc.nc
    xT = nc.dram_tensor("xT", [DIM, T], BF16, kind="ExternalInput").ap()
    w_qk = nc.dram_tensor("w_qk", [DIM, 2 * LOC], BF16, kind="ExternalInput").ap()
    w_v = nc.dram_tensor("w_v", [DIM, LOC], BF16, kind="ExternalInput").ap()
    w_pr = nc.dram_tensor("w_pr", [LOC, DIM], BF16, kind="ExternalInput").ap()
    cosT_d = nc.dram_tensor("cosT", [128, T], BF16, kind="ExternalInput").ap()
    sinT_d = nc.dram_tensor("sinT", [128, T], BF16, kind="ExternalInput").ap()
    prot_d = nc.dram_tensor("prot", [128, 128], BF16, kind="ExternalInput").ap()
    mask01_d = nc.dram_tensor("mask01", [128, 128], BF16, kind="ExternalInput").ap()
    out_d = nc.dram_tensor("out", [T, DIM], BF16, kind="ExternalOutput").ap()

    with (
        tc.tile_pool(name="const", bufs=1) as const,
        tc.tile_pool(name="xt", bufs=2) as xt_pool,
        tc.tile_pool(name="persist", bufs=1) as persist,
        tc.tile_pool(name="work", bufs=4) as work,
        tc.tile_pool(name="expp", bufs=6) as expp,
        tc.tile_pool(name="ps_sc", bufs=2, space="PSUM") as ps_sc,
        tc.tile_pool(name="ps_acc", bufs=1, space="PSUM") as ps_acc,
        tc.tile_pool(name="ps_mm", bufs=1, space="PSUM") as ps_mm,
        tc.tile_pool(name="dram", bufs=2, space="DRAM") as dram,
    ):
        # ---- tb=0 activations + qk weights first: PE starts ASAP ----
        xt_sb = {}
        wqk_sb = []
        for d in range(ND):
            x_t = xt_pool.tile([128, TB], BF16, tag=f"xt{d}", name=f"xt{d}_0")
            nc.sync.dma_start(out=x_t, in_=xT[d * 128:(d + 1) * 128, 0:TB])
            xt_sb[(0, d)] = x_t
            wq_t = const.tile([128, 2 * LOC], BF16, tag=f"wqk{d}", name=f"wqk{d}")
            nc.sync.dma_start(out=wq_t, in_=w_qk[d * 128:(d + 1) * 128, :])
            wqk_sb.append(wq_t)
        prot_sb = const.tile([128, 128], BF16, tag="prot")
        nc.sync.dma_start(out=prot_sb, in_=prot_d)
        cos_sb = const.tile([128, T], BF16, tag="cos")
        nc.sync.dma_start(out=cos_sb, in_=cosT_d)
        sin_sb = const.tile([128, T], BF16, tag="sin")
        nc.sync.dma_start(out=sin_sb, in_=sinT_d)
        wv_sb = []
        for d in range(ND):
            wv_t = const.tile([128, LOC], BF16, tag=f"wv{d}", name=f"wv{d}")
            nc.sync.dma_start(out=wv_t, in_=w_v[d * 128:(d + 1) * 128, :])
            wv_sb.append(wv_t)
        mask_sb = const.tile([128, 128], BF16, tag="mask")
        nc.sync.dma_start(out=mask_sb, in_=mask01_d)
        ones_sb = const.tile([128, 1], BF16, tag="ones")
        nc.vector.memset(ones_sb, 1.0)
        ones2_sb = const.tile([128, 64], F32, tag="ones2")
        nc.vector.memset(ones2_sb, 1.0)
        wpr_sb = []
        for c in range(2):
            wp_t = const.tile([128, DIM], BF16, tag=f"wpr{c}", name=f"wpr{c}")
            nc.sync.dma_start(out=wp_t, in_=w_pr[c * 128:(c + 1) * 128, :])
            wpr_sb.append(wp_t)

        # persistent per-phase outputs
        qk_rope = [[persist.tile([128, TB], BF16, tag=f"qkr{m}_{tb}", name=f"qkr{m}_{tb}")
                    for tb in range(NTB)] for m in range(4)]
        v_sb = [persist.tile([128, LOC], BF16, tag=f"v{ts}", name=f"v{ts}")
                for ts in range(NKT)]

        mask_bc = mask_sb.rearrange("p (o n) -> p o n", o=1).to_broadcast([128, 2, 128])

        hoisted = {}
        for tb in range(NTB):
            # ---- QKV + RoPE for this t block ----
            for m in range(4):  # m 0,1 -> q pairs; 2,3 -> k pairs
                if m == 0 and tb in hoisted:
                    raw_sb = hoisted.pop(tb)
                else:
                    qk1 = (ps_mm.tile([128, TB], F32, tag="mm", name=f"qk1_{m}_{tb}")
                           if m % 2 == 0 else
                           ps_acc.tile([128, TB], F32, tag="av1", name=f"qk1_{m}_{tb}"))
                    for d in range(ND):
                        nc.tensor.matmul(
                            qk1,
                            lhsT=wqk_sb[d][:, m * 128:(m + 1) * 128],
                            rhs=xt_sb[(tb, d)],
                            start=(d == 0), stop=(d == ND - 1),
                        )
                    raw_sb = work.tile([128, TB], BF16, tag="raw")
                    nc.scalar.copy(raw_sb, qk1)
                rot_ps = ps_acc.tile([128, TB], F32, tag="av0", name=f"rot_{m}_{tb}")
                nc.tensor.matmul(rot_ps, lhsT=prot_sb, rhs=raw_sb,
                                 start=True, stop=True)
                qc_sb = work.tile([128, TB], BF16, tag="qc")
                nc.vector.tensor_mul(qc_sb, raw_sb, cos_sb[:, tb * TB:(tb + 1) * TB])
                rs_sb = work.tile([128, TB], BF16, tag="rs")
                nc.vector.tensor_mul(rs_sb, rot_ps, sin_sb[:, tb * TB:(tb + 1) * TB])
                nc.vector.tensor_add(qk_rope[m][tb], qc_sb, rs_sb)
            for s in range(4):
                ts = tb * 4 + s
                v_ps = ps_mm.tile([128, LOC], F32, tag="mm", name=f"v_ps{ts}")
                for d in range(ND):
                    nc.tensor.matmul(
                        v_ps,
                        lhsT=xt_sb[(tb, d)][:, s * 128:(s + 1) * 128],
                        rhs=wv_sb[d],
                        start=(d == 0), stop=(d == ND - 1),
                    )
                nc.scalar.copy(v_sb[ts], v_ps)

            # ---- attention for qb=tb (scoresT layout, flash over kt) ----
            qb = tb
            nkt = 4 * (qb + 1)
            av_ps = [ps_acc.tile([128, TB], F32, tag=f"av{p}", name=f"av{p}_{qb}")
                     for p in range(2)]
            rsum_ps = ps_acc.tile([128, TB], F32, tag="rsum", name=f"rsum_{qb}")
            for kt in range(nkt):
                ktl = kt - 4 * qb
                a = 128 * ktl if ktl >= 0 else 0
                w = TB - a
                tbk, ok = kt // 4, (kt % 4) * 128
                for p in range(2):
                    sc2 = ps_sc.tile([128, 2, TB], F32, tag="sc", name=f"sc{qb}_{kt}_{p}")
                    for j in range(2):
                        nc.tensor.matmul(
                            sc2[:, j, 0:w],
                            lhsT=qk_rope[2 + p][tbk][64 * j:64 * j + 64, ok:ok + 128],
                            rhs=qk_rope[p][qb][64 * j:64 * j + 64, a:TB],
                            start=True, stop=True, tile_position=(64 * j, 0),
                        )
                    exp2 = expp.tile([128, 2, TB], BF16, tag="exp", name=f"exp{qb}_{kt}_{p}")
                    nc.scalar.activation(exp2[:, :, 0:w], sc2[:, :, 0:w],
                                         mybir.ActivationFunctionType.Exp,
                                         scale=float(SCALE))
                    if ktl >= 0:
                        nc.vector.tensor_mul(exp2[:, :, 0:128], exp2[:, :, 0:128],
                                             mask_bc)
                    for j in range(2):
                        h = 2 * p + j
                        nc.tensor.matmul(
                            av_ps[p][64 * j:64 * j + 64, a:TB],
                            lhsT=v_sb[kt][:, 64 * h:64 * h + 64],
                            rhs=exp2[:, j, 0:w],
                            start=(kt == 0), stop=(kt == nkt - 1),
                            skip_group_check=True,
                            tile_position=(0, 64 * j),
                        )
                    for j in range(2):
                        h = 2 * p + j
                        nc.tensor.matmul(
                            rsum_ps[32 * h:32 * h + 1, a:TB],
                            lhsT=ones_sb,
                            rhs=exp2[:, j, 0:w],
                            start=(kt == 0), stop=(kt == nkt - 1),
                            skip_group_check=True,
                            tile_position=(0, 32 * h),
                        )

            # hoist next window's first q projection so the PE has work
            # during the normalizer's DVE latency
            if tb + 1 < NTB:
                for d in range(ND):
                    x_t = xt_pool.tile([128, TB], BF16, tag=f"xt{d}",
                                       name=f"xt{d}_{tb + 1}")
                    nc.sync.dma_start(
                        out=x_t,
                        in_=xT[d * 128:(d + 1) * 128, (tb + 1) * TB:(tb + 2) * TB])
                    xt_sb[(tb + 1, d)] = x_t
                qk1h = ps_mm.tile([128, TB], F32, tag="mm", name=f"qk1_0_{tb + 1}")
                for d in range(ND):
                    nc.tensor.matmul(
                        qk1h,
                        lhsT=wqk_sb[d][:, 0:128],
                        rhs=xt_sb[(tb + 1, d)],
                        start=(d == 0), stop=(d == ND - 1),
                    )
                rawh = work.tile([128, TB], BF16, tag="raw")
                nc.scalar.copy(rawh, qk1h)
                hoisted[tb + 1] = rawh

            # softmax normalizer: copy rowsums to SBUF, replicate via K=1
            # outer-product matmuls, then approx reciprocal on the dense tile
            rsum_sb = work.tile([128, TB], F32, tag="recip")
            nc.vector.tensor_copy(rsum_sb, rsum_ps)
            bc2 = ps_sc.tile([128, 2, TB], F32, tag="sc", name=f"bc{qb}")
            for p in range(2):
                for j in range(2):
                    h = 2 * p + j
                    nc.tensor.matmul(
                        bc2[64 * j:64 * j + 64, p, :],
                        lhsT=ones2_sb[32 * h:32 * h + 1, :],
                        rhs=rsum_sb[32 * h:32 * h + 1, :],
                        start=True, stop=True, skip_group_check=True,
                        tile_position=(32 * h, 64 * j),
                    )
            recip2_sb = work.tile([128, 2, TB], F32, tag="recipb")
            nc.vector.reciprocal_approx_fast(out=recip2_sb, in_=bc2)
            outT_sb = []
            for p in range(2):
                o_t = persist.tile([128, TB], BF16, tag=f"outT{qb}_{p}", name=f"outT{qb}_{p}")
                nc.vector.tensor_mul(o_t, av_ps[p], recip2_sb[:, p, :])
                outT_sb.append(o_t)

            # ---- output projection for this qb's 4 t-subtiles ----
            for s in range(4):
                for n in range(2):
                    pr2 = ps_sc.tile([128, 2, TB], F32, tag="sc", name=f"pr{qb}_{s}_{n}")
                    pr_ps = pr2[:, 0, :]
                    for p in range(2):
                        nc.tensor.matmul(
                            pr_ps,
                            lhsT=outT_sb[p][:, s * 128:(s + 1) * 128],
                            rhs=wpr_sb[p][:, n * TB:(n + 1) * TB],
                            start=(p == 0), stop=(p == 1),
                        )
                    pr_sb = work.tile([128, TB], BF16, tag="pr_sb")
                    nc.vector.tensor_copy(pr_sb, pr_ps)
                    nc.sync.dma_start(
                        out=out_d[(qb * 4 + s) * 128:(qb * 4 + s + 1) * 128,
                                  n * TB:(n + 1) * TB],
                        in_=pr_sb,
                    )


def shard_inputs(x, w_qkv, w_proj):
    """Full inputs -> list of 8 per-core input maps."""
    cosT, sinT, prot, mask01 = _host_constants()
    x = np.ascontiguousarray(np.asarray(x, dtype=np.float32))
    w_qkv = np.asarray(w_qkv, dtype=np.float32)
    w_proj = np.asarray(w_proj, dtype=np.float32)
    in_maps = []
    for c in range(N_CORES):
        b, g = c // TPG, c % TPG
        xT = np.ascontiguousarray(x[b].T)                     # [DIM, T]
        wq = w_qkv[:, g * LOC:(g + 1) * LOC]
        wk = w_qkv[:, INNER + g * LOC:INNER + (g + 1) * LOC]
        wv = w_qkv[:, 2 * INNER + g * LOC:2 * INNER + (g + 1) * LOC]
        w_qk = np.ascontiguousarray(np.concatenate([wq, wk], axis=1))  # [DIM, 512]
        w_pr = np.ascontiguousarray(w_proj[g * LOC:(g + 1) * LOC, :])  # [256, DIM]
        in_maps.append({
            "xT": xT.astype(ml_dtypes.bfloat16),
            "w_qk": w_qk.astype(ml_dtypes.bfloat16),
            "w_v": np.ascontiguousarray(wv).astype(ml_dtypes.bfloat16),
            "w_pr": w_pr.astype(ml_dtypes.bfloat16),
            "cosT": cosT.astype(ml_dtypes.bfloat16),
            "sinT": sinT.astype(ml_dtypes.bfloat16),
            "prot": prot.astype(ml_dtypes.bfloat16),
            "mask01": mask01,
        })
    return in_maps


_CACHE = {}


def _get_compiled():
    if "nc" not in _CACHE:
        nc = bacc.Bacc("TRN2", target_bir_lowering=False, debug=False,
                       enable_asserts=True, num_devices=N_CORES)
        with tile.TileContext(nc) as tc:
            build_kernel(tc)
        nc.compile()
        _CACHE["nc"] = nc
    return _CACHE["nc"]


def kernel(x, w_qkv, w_proj):
    nc = _get_compiled()
    in_maps = shard_inputs(x, w_qkv, w_proj)
    res = run_bass_kernel_spmd(nc, in_maps, core_ids=list(range(N_CORES)))
    outs = [res.results[c]["out"] for c in range(N_CORES)]
    full = np.stack([
        np.sum([outs[b * TPG + g] for g in range(TPG)], axis=0, dtype=np.float32)
        for b in range(B)
    ])
    return full.astype(np.float32)
